# revision 9
# baseline (speedup 1.0000x reference)
"""Trainium2 Bass kernel for nn_DAttention:
out[b,c,d,h,w] = x[b,c,d,h,w] * mean_{c,h,w}(x[b,:,d,:,:]).

Sharding: pure data parallel over batch B=8 -> one batch per NeuronCore.
Numerics: HBM I/O in bf16 (host converts f32->bf16 in, bf16->f32 out);
the mean is accumulated in fp32, so element error is ~2 bf16 roundings
(~0.4%) -- far inside the 2e-2 gate. 32 MiB in + 32 MiB out per core.

DMA load-balancing: an InstDMACopy's descriptors are dealt to the 16
SDMA engines in consecutive chunks of ceil(n_desc/16) (measured: a
128-row DMA gives every engine 8 rows; a 120-row DMA gives engines
0-14 eight rows and engine 15 none; an 8-row DMA lands on engines 0-7
one each). Engine 15 runs ~15% slower than the rest under load
(fat-tailed packet durations, HW arbitration) and is the critical path
of a uniform 128-row layout. So each 2^19-element d-slice is dealt
(host-side) into 127 rows of 4128 elements (8256 B, 64B-aligned
descriptors): engines 0-14 carry 8 rows each, engine 15 carries 7 --
a 12.5% relief that absorbs its deficit. The 32-element remainder per
slice is off the hot path entirely: the host ships its fp32 sum as a
tiny input (folded into the mean by a third matmul), and the kernel
returns the 32 per-slice means (128 B) so the host scales those 32
elements itself (0.006% of the work).

Per-slice schedule:
  ACT: activation-Copy (dead PSUM scratch) with accum_out -> fp32
       column sums of xt[:, :A]
  DVE: tensor_reduce(add) -> fp32 column sums of xt[:, A:]
  PE : three accumulated fp32 matmuls (vs a 128x128 constant 1/2^19
       matrix, plus the remainder-sum term) -> mean broadcast [128,1]
  ACT: tiny copies mean PSUM->SBUF (multiply operand + means output)
  DVE: one tensor_scalar multiply (bf16, ~4 elem/cyc/lane) full tile
  DMA: loads on the SP HWDGE ring, stores on the ACT ring.
Row 127 of each pool buffer is zeroed once (memset from partition 96;
loads overwrite 96-126) so full-128-row reductions stay exact.
"""
import numpy as np
import ml_dtypes

import concourse.bacc as bacc
import concourse.tile as tile
import concourse.mybir as mybir
from concourse.bass_utils import run_bass_kernel_spmd

BF16 = ml_dtypes.bfloat16

B, C, D, H, W = 8, 32, 32, 128, 128
N = C * H * W           # 524288 = 2**19 elements per (b, d) slice
RECIP = 1.0 / N         # exact in fp32
PR = 127                # rows per slice
FO = 4128               # row width (8256 B, 64B-aligned)
LM = PR * FO            # 524256 elems in the main rows
REM = N - LM            # 32 remainder elems per slice
A_SPLIT = 2688          # ACT reduces xt[:, :A], DVE reduces xt[:, A:]
XIN_BUFS = 6

_NC = None


def _build_nc(xin_bufs=XIN_BUFS, out_bufs=3):
    nc = bacc.Bacc("TRN2", target_bir_lowering=False, debug=False)
    xa = nc.dram_tensor("xa", [D, PR, FO], mybir.dt.bfloat16, kind="ExternalInput")
    xr = nc.dram_tensor("xr", [1, D], mybir.dt.float32, kind="ExternalInput")
    oa = nc.dram_tensor("oa", [D, PR, FO], mybir.dt.bfloat16, kind="ExternalOutput")
    om = nc.dram_tensor("om", [1, D], mybir.dt.float32, kind="ExternalOutput")
    with tile.TileContext(nc) as tc:
        with (
            tc.tile_pool(name="xin", bufs=xin_bufs) as xpool,
            tc.tile_pool(name="oout", bufs=out_bufs) as opool,
            tc.tile_pool(name="small", bufs=6) as spool,
            tc.tile_pool(name="psum", bufs=1, space="PSUM") as ppool,
            tc.tile_pool(name="psc", bufs=1, space="PSUM") as scpool,
            tc.tile_pool(name="const", bufs=1) as cpool,
        ):
            recip = cpool.tile([128, 128], mybir.dt.float32)
            nc.gpsimd.memset(recip[:], RECIP)
            rr = cpool.tile([1, 128], mybir.dt.float32)
            nc.gpsimd.memset(rr[:], RECIP)
            xrs = cpool.tile([1, D], mybir.dt.float32)
            nc.sync.dma_start(xrs[:], xr[:])
            meansrow = cpool.tile([1, D], mybir.dt.float32)

            for d in range(D):
                xt = xpool.tile([128, FO], mybir.dt.bfloat16, tag="xt")
                if d < xin_bufs:
                    # zero row 127 once per physical buffer (engine ops
                    # need partition start % 32 == 0; rows 96-126 get
                    # overwritten by the load right after)
                    nc.vector.memset(xt[96:, :], 0.0)
                nc.sync.dma_start(xt[:PR, :], xa[d])
                csa = spool.tile([128, 1], mybir.dt.float32, tag="csa")
                csb = spool.tile([128, 1], mybir.dt.float32, tag="csb")
                scrA = scpool.tile([128, A_SPLIT], mybir.dt.float32, tag="scA")
                nc.scalar.activation(
                    scrA[:], xt[:, :A_SPLIT],
                    mybir.ActivationFunctionType.Copy, accum_out=csa[:],
                )
                nc.vector.tensor_reduce(
                    csb[:], xt[:, A_SPLIT:],
                    mybir.AxisListType.X, mybir.AluOpType.add,
                )
                dv = ppool.tile([128, 1], mybir.dt.float32, tag="dv")
                nc.tensor.matmul(dv[:], recip[:], csa[:], start=True, stop=False)
                nc.tensor.matmul(dv[:], recip[:], csb[:], start=False, stop=False)
                nc.tensor.matmul(dv[:], rr[:], xrs[:, d : d + 1], start=False, stop=True)
                dvs = spool.tile([128, 1], mybir.dt.float32, tag="dvs")
                nc.scalar.copy(dvs[:], dv[:])
                nc.scalar.copy(meansrow[:, d : d + 1], dv[:1, :])
                ot = opool.tile([128, FO], mybir.dt.bfloat16, tag="ot")
                nc.vector.tensor_scalar_mul(ot[:], xt[:], dvs[:])
                nc.scalar.dma_start(oa[d], ot[:PR, :])
            nc.sync.dma_start(om[:], meansrow[:])
    nc.compile()
    return nc


def _get_nc():
    global _NC
    if _NC is None:
        _NC = _build_nc()
    return _NC


def _deal_in(x_core: np.ndarray):
    """[C,D,H,W] f32 -> (xa [D,PR,FO] bf16, xr [1,D] f32, xrem [D,REM] bf16)."""
    xd = np.ascontiguousarray(x_core.astype(BF16).transpose(1, 0, 2, 3)).reshape(D, N)
    xa = np.ascontiguousarray(xd[:, :LM]).reshape(D, PR, FO)
    xrem = np.ascontiguousarray(xd[:, LM:])               # [D, REM] bf16
    xr = xrem.astype(np.float32).sum(axis=1).reshape(1, D)
    return xa, xr, xrem


def _deal_out(oa_core: np.ndarray, om_core: np.ndarray, xrem: np.ndarray):
    """(oa [D,PR,FO] bf16, om [1,D] f32 means) -> [C,D,H,W] f32."""
    od = np.empty((D, N), BF16)
    od[:, :LM] = oa_core.reshape(D, LM)
    od[:, LM:] = (xrem.astype(np.float32) * om_core.reshape(D, 1)).astype(BF16)
    return od.reshape(D, C, H, W).transpose(1, 0, 2, 3).astype(np.float32)


def run(x: np.ndarray, trace: bool = False, tmpdir: str | None = None):
    """Run on 8 NeuronCores; returns (out, BassKernelResults)."""
    x = np.asarray(x)
    assert x.shape == (B, C, D, H, W), x.shape
    nc = _get_nc()
    in_maps, xrems = [], []
    for b in range(B):
        xa, xr, xrem = _deal_in(x[b])
        in_maps.append({"xa": xa, "xr": xr})
        xrems.append(xrem)
    res = run_bass_kernel_spmd(
        nc, in_maps, core_ids=list(range(B)), trace=trace, tmpdir=tmpdir
    )
    out = np.stack(
        [_deal_out(r["oa"], r["om"], xrems[b]) for b, r in enumerate(res.results)]
    )
    return out, res


def kernel(x: np.ndarray) -> np.ndarray:
    out, _ = run(x)
    return out


# revision 10
# speedup vs baseline: 11.4048x; 11.4048x over previous
"""Trainium2 Bass kernel for nn_DAttention:
out[b,c,d,h,w] = x[b,c,d,h,w] * mean_{c,h,w}(x[b,:,d,:,:]).

Sharding: pure data parallel over batch B=8 -> one batch per NeuronCore.
Numerics: HBM I/O in bf16 (host converts f32->bf16 in, bf16->f32 out);
the mean is accumulated in fp32, so element error is ~2 bf16 roundings
(~0.4%) -- far inside the 2e-2 gate. 32 MiB in + 32 MiB out per core.

DMA load-balancing: an InstDMACopy's descriptors are dealt to the 16
SDMA engines in consecutive chunks of ceil(n_rows/16), but only when
n_rows splits evenly (measured: 128 rows -> 8/engine on all 16;
120 rows -> 8/engine on engines 0-14, engine 15 idle; 8 rows ->
1/engine on engines 0-7; 127 rows falls off the balancer and lands on
ONE engine). Engine 15 runs ~15% slower than the rest under load
(fat-tailed packet durations, HW arbitration) and is the critical path
of a uniform layout. So each 2^19-element d-slice is two DMAs into one
[128, 4128] tile: [128 rows x 3616 cols] (engine 15 included) plus
[120 rows x 512 cols] at cols 3616:4128 (engines 0-14 only):
128*3616 + 120*512 = 2^19 exactly, engine 15 carries 87.6% of the
others' bytes, absorbing its deficit; every descriptor is 64B-aligned
(7232 B / 1024 B). The [120:128, 3616:4128] pad strip of each pool
buffer is zeroed once so full-rectangle reductions stay exact.

Per-slice schedule:
  ACT: activation-Copy (dead PSUM scratch) with accum_out -> fp32
       column sums of xt[:, :A]
  DVE: tensor_reduce(add) -> fp32 column sums of xt[:, A:]
  PE : two accumulated fp32 matmuls vs a 128x128 constant matrix of
       1/2^19 -> total mean broadcast to [128,1] PSUM
  ACT: tiny copy mean PSUM->SBUF
  DVE: one tensor_scalar multiply (bf16, ~4 elem/cyc/lane) full tile
  DMA: loads on the SP HWDGE ring, stores on the ACT ring.
"""
import numpy as np
import ml_dtypes

import concourse.bacc as bacc
import concourse.tile as tile
import concourse.mybir as mybir
from concourse.bass_utils import run_bass_kernel_spmd

BF16 = ml_dtypes.bfloat16

B, C, D, H, W = 8, 32, 32, 128, 128
N = C * H * W           # 524288 = 2**19 elements per (b, d) slice
RECIP = 1.0 / N         # exact in fp32
F1 = 3616               # all-128-partition region width (7232 B rows)
F2 = 512                # extra region width, partitions 0-119 (1024 B)
FO = F1 + F2            # 4128-wide SBUF tile
PM = 120
L1 = 128 * F1           # 462848
assert 128 * F1 + PM * F2 == N
A_SPLIT = 2304          # ACT reduces xt[:, :A], DVE reduces xt[:, A:]
XIN_BUFS = 6

_NC = None


def _build_nc(xin_bufs=XIN_BUFS, out_bufs=3):
    nc = bacc.Bacc("TRN2", target_bir_lowering=False, debug=False)
    xa1 = nc.dram_tensor("xa1", [D, 128, F1], mybir.dt.bfloat16, kind="ExternalInput")
    xa2 = nc.dram_tensor("xa2", [D, PM, F2], mybir.dt.bfloat16, kind="ExternalInput")
    oa1 = nc.dram_tensor("oa1", [D, 128, F1], mybir.dt.bfloat16, kind="ExternalOutput")
    oa2 = nc.dram_tensor("oa2", [D, PM, F2], mybir.dt.bfloat16, kind="ExternalOutput")
    with tile.TileContext(nc) as tc:
        with (
            tc.tile_pool(name="xin", bufs=xin_bufs) as xpool,
            tc.tile_pool(name="oout", bufs=out_bufs) as opool,
            tc.tile_pool(name="small", bufs=6) as spool,
            tc.tile_pool(name="psum", bufs=1, space="PSUM") as ppool,
            tc.tile_pool(name="psc", bufs=1, space="PSUM") as scpool,
            tc.tile_pool(name="const", bufs=1) as cpool,
        ):
            recip = cpool.tile([128, 128], mybir.dt.float32)
            nc.gpsimd.memset(recip[:], RECIP)

            for d in range(D):
                xt = xpool.tile([128, FO], mybir.dt.bfloat16, tag="xt")
                if d < xin_bufs:
                    # zero the pad strip once per physical buffer (engine
                    # ops need partition start % 32 == 0; rows 96-119 get
                    # overwritten by the second load right after)
                    nc.vector.memset(xt[96:, F1:], 0.0)
                nc.sync.dma_start(xt[:, :F1], xa1[d])
                nc.sync.dma_start(xt[:PM, F1:], xa2[d])
                csa = spool.tile([128, 1], mybir.dt.float32, tag="csa")
                csb = spool.tile([128, 1], mybir.dt.float32, tag="csb")
                scrA = scpool.tile([128, A_SPLIT], mybir.dt.float32, tag="scA")
                nc.scalar.activation(
                    scrA[:], xt[:, :A_SPLIT],
                    mybir.ActivationFunctionType.Copy, accum_out=csa[:],
                )
                nc.vector.tensor_reduce(
                    csb[:], xt[:, A_SPLIT:],
                    mybir.AxisListType.X, mybir.AluOpType.add,
                )
                dv = ppool.tile([128, 1], mybir.dt.float32, tag="dv")
                nc.tensor.matmul(dv[:], recip[:], csa[:], start=True, stop=False)
                nc.tensor.matmul(dv[:], recip[:], csb[:], start=False, stop=True)
                dvs = spool.tile([128, 1], mybir.dt.float32, tag="dvs")
                nc.scalar.copy(dvs[:], dv[:])
                ot = opool.tile([128, FO], mybir.dt.bfloat16, tag="ot")
                nc.vector.tensor_scalar_mul(ot[:], xt[:], dvs[:])
                nc.scalar.dma_start(oa1[d], ot[:, :F1])
                nc.scalar.dma_start(oa2[d], ot[:PM, F1:])
    nc.compile()
    return nc


def _get_nc():
    global _NC
    if _NC is None:
        _NC = _build_nc()
    return _NC


def _deal_in(x_core: np.ndarray):
    """[C,D,H,W] f32 -> (xa1 [D,128,F1], xa2 [D,PM,F2]) bf16."""
    xd = np.ascontiguousarray(x_core.astype(BF16).transpose(1, 0, 2, 3)).reshape(D, N)
    xa1 = np.ascontiguousarray(xd[:, :L1]).reshape(D, 128, F1)
    xa2 = np.ascontiguousarray(xd[:, L1:]).reshape(D, PM, F2)
    return xa1, xa2


def _deal_out(oa1_core: np.ndarray, oa2_core: np.ndarray):
    """(oa1 [D,128,F1], oa2 [D,PM,F2]) bf16 -> [C,D,H,W] f32."""
    od = np.empty((D, N), BF16)
    od[:, :L1] = oa1_core.reshape(D, L1)
    od[:, L1:] = oa2_core.reshape(D, PM * F2)
    return od.reshape(D, C, H, W).transpose(1, 0, 2, 3).astype(np.float32)


def run(x: np.ndarray, trace: bool = False, tmpdir: str | None = None):
    """Run on 8 NeuronCores; returns (out, BassKernelResults)."""
    x = np.asarray(x)
    assert x.shape == (B, C, D, H, W), x.shape
    nc = _get_nc()
    in_maps = []
    for b in range(B):
        xa1, xa2 = _deal_in(x[b])
        in_maps.append({"xa1": xa1, "xa2": xa2})
    res = run_bass_kernel_spmd(
        nc, in_maps, core_ids=list(range(B)), trace=trace, tmpdir=tmpdir
    )
    out = np.stack([_deal_out(r["oa1"], r["oa2"]) for r in res.results])
    return out, res


def kernel(x: np.ndarray) -> np.ndarray:
    out, _ = run(x)
    return out


# revision 11
# speedup vs baseline: 12.6652x; 1.1105x over previous
"""Trainium2 Bass kernel for nn_DAttention:
out[b,c,d,h,w] = x[b,c,d,h,w] * mean_{c,h,w}(x[b,:,d,:,:]).

Sharding: pure data parallel over batch B=8 -> one batch per NeuronCore
(x[b] is a contiguous zero-copy slice). Numerics: HBM I/O in bf16 (the
host converts f32->bf16 on the way in and bf16->f32 on the way out);
the mean is accumulated in fp32 (ACT accum_out + fp32 matmuls), so the
element error is ~2 bf16 roundings (~0.4%) -- far inside the 2e-2
gate. This halves HBM traffic vs f32: 32 MiB in + 32 MiB out per core,
the memory-roofline minimum for this regime.

Per core, loop over the 32 d-slices (1 MiB each in bf16): load
x[b,:,d,:,:] into SBUF, reduce to the scalar mean, multiply, store.

SBUF layout per d-slice: tile [128, 4096] bf16 with partition
p = c*4 + hg (H split into 4 groups of 32), free = (h%32)*128 + w.
Each partition row is one contiguous 8192-byte DRAM run -> every DMA
descriptor is a power-of-2 8 KiB (measured: 8/16 KiB descriptors run
at line rate +8 ns; odd sizes like 6.3/7.1 KiB pay +30-80 ns each, and
non-multiple-of-8 row counts fall off the 16-engine descriptor
balancer entirely).

Engine schedule per slice (balanced so neither ACT nor DVE paces the
post-load tail far above the store-DMA rate):
  ACT: activation-Copy of xt[:, :A] into a dead PSUM scratch with
       accum_out -> fp32 column sums (the PSUM write is free bandwidth;
       only Matmul/Memset may write bf16 to PSUM, so scratch is f32)
  DVE: tensor_reduce(add) of xt[:, A:] -> fp32 column sums
  PE : two accumulated fp32 matmuls against a constant 128x128 matrix
       of 1/2^19 -> cross-partition sum + broadcast of the mean to all
       partitions in one PSUM [128,1]
  ACT: tiny copy of the mean PSUM->SBUF
  DVE: single tensor_scalar multiply (bf16, ~4 elem/cyc/lane)
  DMA: loads on the SP HWDGE ring, stores on the ACT ring
"""
import numpy as np
import ml_dtypes

import concourse.bacc as bacc
import concourse.tile as tile
import concourse.mybir as mybir
from concourse.bass_utils import run_bass_kernel_spmd

BF16 = ml_dtypes.bfloat16

B, C, D, H, W = 8, 32, 32, 128, 128
HG, HL = 4, 32          # H split: partition dim = C*HG = 128
P = C * HG              # 128 partitions
F = HL * W              # 4096 free elements per partition
N_RED = C * H * W       # 524288 = 2**19 elements reduced per (b, d)
RECIP = 1.0 / N_RED     # exact in fp32
A_SPLIT = 2432          # ACT reduces xt[:, :A], DVE reduces xt[:, A:]

_NC = None


def _build_nc(xin_bufs=12, out_bufs=4):
    nc = bacc.Bacc("TRN2", target_bir_lowering=False, debug=False)
    x5 = nc.dram_tensor("x", [C, D, HG, HL, W], mybir.dt.bfloat16, kind="ExternalInput")
    o5 = nc.dram_tensor("out", [C, D, HG, HL, W], mybir.dt.bfloat16, kind="ExternalOutput")
    with tile.TileContext(nc) as tc:
        with (
            tc.tile_pool(name="xin", bufs=xin_bufs) as xpool,
            tc.tile_pool(name="oout", bufs=out_bufs) as opool,
            tc.tile_pool(name="small", bufs=6) as spool,
            tc.tile_pool(name="psum", bufs=2, space="PSUM") as ppool,
            tc.tile_pool(name="psc", bufs=1, space="PSUM") as scpool,
            tc.tile_pool(name="const", bufs=1) as cpool,
        ):
            recip = cpool.tile([P, P], mybir.dt.float32)
            nc.gpsimd.memset(recip[:], RECIP)
            for d in range(D):
                xt = xpool.tile([P, F], mybir.dt.bfloat16, tag="xt")
                nc.sync.dma_start(xt[:], x5[:, d])
                csa = spool.tile([P, 1], mybir.dt.float32, tag="csa")
                csb = spool.tile([P, 1], mybir.dt.float32, tag="csb")
                scratch = scpool.tile([P, A_SPLIT], mybir.dt.float32, tag="sc")
                nc.scalar.activation(
                    scratch[:], xt[:, :A_SPLIT],
                    mybir.ActivationFunctionType.Copy, accum_out=csa[:],
                )
                nc.vector.tensor_reduce(
                    csb[:], xt[:, A_SPLIT:],
                    mybir.AxisListType.X, mybir.AluOpType.add,
                )
                dv = ppool.tile([P, 1], mybir.dt.float32, tag="dv")
                nc.tensor.matmul(dv[:], recip[:], csa[:], start=True, stop=False)
                nc.tensor.matmul(dv[:], recip[:], csb[:], start=False, stop=True)
                dvs = spool.tile([P, 1], mybir.dt.float32, tag="dvs")
                nc.scalar.copy(dvs[:], dv[:])
                ot = opool.tile([P, F], mybir.dt.bfloat16, tag="ot")
                nc.vector.tensor_scalar_mul(ot[:], xt[:], dvs[:])
                nc.scalar.dma_start(o5[:, d], ot[:])
    nc.compile()
    return nc


def _get_nc():
    global _NC
    if _NC is None:
        _NC = _build_nc()
    return _NC


def run(x: np.ndarray, trace: bool = False, tmpdir: str | None = None):
    """Run on 8 NeuronCores; returns (out, BassKernelResults)."""
    x = np.asarray(x)
    assert x.shape == (B, C, D, H, W), x.shape
    xb = x.astype(BF16)
    nc = _get_nc()
    in_maps = [
        {"x": np.ascontiguousarray(xb[b]).reshape(C, D, HG, HL, W)} for b in range(B)
    ]
    res = run_bass_kernel_spmd(
        nc, in_maps, core_ids=list(range(B)), trace=trace, tmpdir=tmpdir
    )
    out = np.stack(
        [r["out"].astype(np.float32).reshape(C, D, H, W) for r in res.results]
    )
    return out, res


def kernel(x: np.ndarray) -> np.ndarray:
    out, _ = run(x)
    return out
